# revision 14
# baseline (speedup 1.0000x reference)
"""Trainium2 Bass kernel for VITS-style relative-position MultiHeadAttention.

Problem: B=4, T=1024, C=512, H=8 heads, d=64, window=4 relative attention
(rel embeddings shared across heads). Sharded over 8 NeuronCores as
(batch x head-group): core = 2*b + hg, each core handles batch b and 4 heads.

v4 changes vs v3 (trace-driven):
  - warmup matmul burst + exp-table preload at t=0 (HAM stays at K=8/8)
  - DMA order wq+xT first; q projection starts as tiles land (kt-outer),
    rel-K skew bounce batched into 8 DMAs on the scalar HWDGE ring and
    overlapped with the k/v projections (was a 16.8us PE stall)
  - et / vaug / G-band storage in bf16 (halves band DMA + DVE traffic)
  - rel-V band read (abt) pulled straight out of the skewed G buffer with
    a diagonal access pattern: the per-head abs4 readback, PE transposes,
    at_cat copy and atd re-skew of v3 are gone entirely
  - softmax reciprocal on [128,16]-shaped tiles via SBUF->SBUF DMA
    reshape (was 7.8us per pair on [2,1024] = 2 DVE lanes)
  - pair-0 normalization + outT multiply run mid-flight under heads 2/3
"""

import numpy as np

import concourse.bass as bass
import concourse.bacc as bacc
import concourse.mybir as mybir
import concourse.tile as tile
from concourse.bass_utils import run_bass_kernel_spmd
from concourse.masks import make_identity

f32 = mybir.dt.float32
f32r = mybir.dt.float32r
bf16 = mybir.dt.bfloat16
i16 = mybir.dt.int16

T = 1024          # sequence length (t_t == t_s)
CIN = 512         # input channels
CH = 256          # channels per core (head group)
NHEADS = 4        # heads per core
D = 64            # head dim
NB = 9            # band width (2*window+1)
NT = T // 128     # 8 tiles of 128
GPITCH = 137      # G buffer row pitch (136 + 1)
GBASE = 4 * GPITCH          # origin shift: row s lives at GBASE + s*GPITCH
GSZ = (T + 8) * GPITCH + 32  # rows -4 .. 1027 plus slack
RLDW = T + 8      # rld row pitch

Exp = mybir.ActivationFunctionType.Exp
Identity = mybir.ActivationFunctionType.Identity
Copy = mybir.ActivationFunctionType.Copy
AluAdd = mybir.AluOpType.add
AluMult = mybir.AluOpType.mult


def build_program():
    nc = bacc.Bacc()

    # fp32r matmul: fp32 data, 1 PE cycle/row when moving dim >= 256
    def mmr(out, lhsT, rhs, **kw):
        nc.tensor.matmul(out, lhsT.bitcast(f32r), rhs.bitcast(f32r), **kw)

    def trp(out, in_, identity):
        nc.tensor.matmul(out, in_, identity, is_transpose=True)

    # ---- external I/O (per-core shapes) ----
    xT = nc.declare_dram_parameter("xT", [CIN, T], f32r, isOutput=False)
    cT = nc.declare_dram_parameter("cT", [CIN, T], f32r, isOutput=False)
    wq = nc.declare_dram_parameter("wq", [CIN, CH], f32r, isOutput=False)
    wk = nc.declare_dram_parameter("wk", [CIN, CH], f32r, isOutput=False)
    wv = nc.declare_dram_parameter("wv", [CIN, CH], f32r, isOutput=False)
    wo = nc.declare_dram_parameter("wo", [CH, CIN], f32r, isOutput=False)
    bq2 = nc.declare_dram_parameter("bq2", [128, 2], f32, isOutput=False)
    bk2 = nc.declare_dram_parameter("bk2", [128, 2], f32, isOutput=False)
    bv1 = nc.declare_dram_parameter("bv1", [1, CH], f32r, isOutput=False)
    ek2p = nc.declare_dram_parameter("ek2p", [128, 32 + NB], f32r, isOutput=False)
    ev65 = nc.declare_dram_parameter("ev65", [NB, D + 1], f32, isOutput=False)
    ones128 = nc.declare_dram_parameter("ones128", [1, 128], f32r, isOutput=False)
    e2p = nc.declare_dram_parameter("e2p", [2, 128], f32r, isOutput=False)
    sidx = nc.declare_dram_parameter("sidx", [128, 10], i16, isOutput=False)
    sidxA = nc.declare_dram_parameter("sidxA", [128, 136], i16, isOutput=False)
    sidxB = nc.declare_dram_parameter("sidxB", [128, 136], i16, isOutput=False)
    out_p = nc.declare_dram_parameter("out_p", [T, CIN], f32, isOutput=True)

    with tile.TileContext(nc) as tc:
        with (
            tc.tile_pool(name="const", bufs=1) as cpool,
            tc.tile_pool(name="win", bufs=1) as wpool,
            tc.tile_pool(name="xin", bufs=1) as xpool,
            tc.tile_pool(name="qk", bufs=1) as qkpool,
            tc.tile_pool(name="vaug", bufs=1) as vpool,
            tc.tile_pool(name="band", bufs=1) as bpool,
            tc.tile_pool(name="et", bufs=8) as etpool,
            tc.tile_pool(name="outp", bufs=1) as opool,
            tc.tile_pool(name="dram", bufs=1, space="DRAM") as dpool,
        ):
            # ---------- constants (scalar HWDGE ring) ----------
            ident = cpool.tile([128, 128], f32)
            make_identity(nc, ident[:])
            identb = cpool.tile([128, 128], bf16)
            make_identity(nc, identb[:])
            ones1 = cpool.tile([1, 128], f32r)
            nc.scalar.dma_start(ones1[:], ones128[:])
            e2_sb = cpool.tile([2, 128], f32r)
            nc.scalar.dma_start(e2_sb[:], e2p[:])
            sidx_sb = cpool.tile([128, 10], i16)
            nc.scalar.dma_start(sidx_sb[:], sidx[:])
            sidxA_sb = cpool.tile([128, 136], i16)
            nc.scalar.dma_start(sidxA_sb[:], sidxA[:])
            sidxB_sb = cpool.tile([128, 136], i16)
            nc.scalar.dma_start(sidxB_sb[:], sidxB[:])
            ek2 = cpool.tile([128, 32 + NB], f32r)
            nc.scalar.dma_start(ek2[:], ek2p[:])
            ev_f = cpool.tile([NB, D + 1], f32)
            nc.scalar.dma_start(ev_f[:], ev65[:])
            ev_sb = cpool.tile([NB, D + 1], bf16)
            nc.vector.tensor_copy(ev_sb[:], ev_f[:])
            bq_sb = cpool.tile([128, 2], f32)
            nc.scalar.dma_start(bq_sb[:], bq2[:])
            bk_sb = cpool.tile([128, 2], f32)
            nc.scalar.dma_start(bk_sb[:], bk2[:])
            bv_sb = cpool.tile([1, CH], f32r)
            nc.scalar.dma_start(bv_sb[:], bv1[:])
            zb16 = cpool.tile([NB, 8], bf16)
            nc.gpsimd.memset(zb16[:], 0.0)
            zb_f = cpool.tile([36, 8], f32)
            nc.gpsimd.memset(zb_f[:], 0.0)

            # rld bounce (rel-K skew) borders: cols 0..3 and T+4..T+7 of all
            # 36 rows
            rld = dpool.tile([1, 36 * RLDW], f32, name="rld")
            nc.scalar.dma_start(
                bass.AP(rld[:].tensor, rld[:].offset,
                        [[RLDW, 36], [T + 4, 2], [1, 4]]),
                bass.AP(zb_f[:].tensor, zb_f[:].offset, [[8, 36], [4, 2], [1, 4]]),
            )
            # atd bounce (rel-V skew), bf16, rows j=0..8 pitch T+8
            atd = dpool.tile([1, NB * RLDW], bf16, name="atd")
            nc.scalar.dma_start(
                bass.AP(atd[:].tensor, atd[:].offset,
                        [[RLDW, NB], [T + 4, 2], [1, 4]]),
                bass.AP(zb16[:].tensor, zb16[:].offset, [[8, NB], [4, 2], [1, 4]]),
            )

            # ---------- PE warmup (HAM) + ACT exp-table preload ----------
            wact = cpool.tile([1, 2], f32)
            nc.scalar.activation(wact[0:1, 0:1], ident[0:1, 0:1], Exp)
            with tc.tile_pool(name="psW", bufs=1, space="PSUM") as psW:
                wps = psW.tile([128, 128], f32)
                for _ in range(8):
                    nc.tensor.matmul(wps[:], ident[:], ident[:],
                                     start=True, stop=True)

            # ---------- input loads (sync ring), q-critical first ----------
            wq_sb = []
            xT_sb = []
            for kt in range(4):
                t_ = wpool.tile([128, CH], f32r, tag=f"wq{kt}")
                nc.sync.dma_start(t_[:], wq[kt * 128:(kt + 1) * 128, :])
                wq_sb.append(t_)
                t_ = xpool.tile([128, T], f32r, tag=f"xT{kt}")
                nc.sync.dma_start(t_[:], xT[kt * 128:(kt + 1) * 128, :])
                xT_sb.append(t_)
            wk_sb = []
            cT_sb = []
            for kt in range(4):
                t_ = wpool.tile([128, CH], f32r, tag=f"wk{kt}")
                nc.scalar.dma_start(t_[:], wk[kt * 128:(kt + 1) * 128, :])
                wk_sb.append(t_)
                t_ = xpool.tile([128, T], f32r, tag=f"cT{kt}")
                nc.scalar.dma_start(t_[:], cT[kt * 128:(kt + 1) * 128, :])
                cT_sb.append(t_)
            wv_sb = []
            for kt in range(4):
                t_ = wpool.tile([128, CH], f32r, tag=f"wv{kt}")
                nc.sync.dma_start(t_[:], wv[kt * 128:(kt + 1) * 128, :])
                wv_sb.append(t_)
            wo_sb = []
            for ct in range(2):
                t_ = wpool.tile([128, CIN], f32r, tag=f"wo{ct}")
                nc.sync.dma_start(t_[:], wo[ct * 128:(ct + 1) * 128, :])
                wo_sb.append(t_)

            # band-prep SBUF tiles (memset before skew readback writes rows)
            rlp_cat = bpool.tile([128, T], f32, tag="rlpc")
            s4t_cat = bpool.tile([64, T], f32, tag="s4t")
            nc.gpsimd.memset(s4t_cat[:], 0.0)
            sbf_all = bpool.tile([128, NT * NHEADS * 10], bf16, tag="sbfall")
            nc.gpsimd.memset(sbf_all[:], 0.0)

            qsT_sb = [qkpool.tile([128, T], f32r, tag=f"qsT{ct}", name=f"qsT{ct}")
                      for ct in range(2)]
            kT_sb = [qkpool.tile([128, T], f32r, tag=f"kT{ct}", name=f"kT{ct}")
                     for ct in range(2)]

            with tc.tile_pool(name="psAB", bufs=1, space="PSUM") as psAB:
                # ---- q projection, kt-outer so matmuls start on first tiles
                psq = {}
                for ct in range(2):
                    for nh in range(2):
                        psq[(ct, nh)] = psAB.tile(
                            [128, 512], f32, tag=f"q{ct}{nh}", bufs=1,
                            name=f"psq{ct}{nh}")
                for kt in range(4):
                    for ct in range(2):
                        for nh in range(2):
                            mmr(
                                psq[(ct, nh)][:],
                                wq_sb[kt][:, ct * 128:(ct + 1) * 128],
                                xT_sb[kt][:, nh * 512:(nh + 1) * 512],
                                start=(kt == 0), stop=(kt == 3),
                            )
                for ct in range(2):
                    for nh in range(2):
                        # q_scaled = (x@Wq)*0.125 + bq*0.125 (bq2 pre-scaled)
                        nc.scalar.activation(
                            qsT_sb[ct][:, nh * 512:(nh + 1) * 512],
                            psq[(ct, nh)][:],
                            Identity, bias=bq_sb[:, ct:ct + 1], scale=0.125,
                        )

                # ---- rel-K band logits + batched skew bounce (scalar ring)
                for ct in range(2):
                    for nh in range(2):
                        rlt = psAB.tile([41, 512], f32, tag="small", bufs=2)
                        mmr(rlt[:], ek2[:],
                            qsT_sb[ct][:, nh * 512:(nh + 1) * 512],
                            start=True, stop=True)
                        for hh in range(2):
                            h = 2 * ct + hh
                            nc.vector.tensor_copy(
                                rlp_cat[h * 32:h * 32 + NB,
                                        nh * 512:(nh + 1) * 512],
                                rlt[hh * 32:hh * 32 + NB, :],
                            )
                for h in range(NHEADS):
                    nc.sync.dma_start(
                        bass.AP(rld[:].tensor,
                                rld[:].offset + h * 9 * RLDW + 4,
                                [[RLDW, NB], [1, T]]),
                        rlp_cat[h * 32:h * 32 + NB, :],
                    )
                for h in range(NHEADS):
                    # s4t'[h*16+r, c] = rld[h*9+r, 8-r+c] (pitch T+7 re-read)
                    nc.sync.dma_start(
                        s4t_cat[h * 16:h * 16 + NB, :],
                        bass.AP(rld[:].tensor,
                                rld[:].offset + h * 9 * RLDW + 8,
                                [[T + 7, NB], [1, T]]),
                    )

                # ---- k projection (reuses the q psum tags)
                psk = {}
                for ct in range(2):
                    for nh in range(2):
                        psk[(ct, nh)] = psAB.tile(
                            [128, 512], f32, tag=f"q{ct}{nh}", bufs=1,
                            name=f"psk{ct}{nh}")
                for kt in range(4):
                    for ct in range(2):
                        for nh in range(2):
                            mmr(
                                psk[(ct, nh)][:],
                                wk_sb[kt][:, ct * 128:(ct + 1) * 128],
                                cT_sb[kt][:, nh * 512:(nh + 1) * 512],
                                start=(kt == 0), stop=(kt == 3),
                            )
                for ct in range(2):
                    for nh in range(2):
                        nc.vector.tensor_scalar(
                            kT_sb[ct][:, nh * 512:(nh + 1) * 512],
                            psk[(ct, nh)][:],
                            bk_sb[:, ct:ct + 1], None, op0=AluAdd,
                        )

                # ---- transpose skewed rel-K logits into S layout
                pst = psAB.tile([128, 512], f32, tag="small", bufs=2)
                for st in range(NT):
                    trp(
                        pst[:, st * 64:(st + 1) * 64],
                        s4t_cat[:, st * 128:(st + 1) * 128],
                        ident[0:64, 0:64],
                    )
                nc.vector.tensor_copy(
                    sbf_all[:].rearrange("p (g c) -> p g c", g=32)[:, :, 0:NB],
                    pst[:].rearrange("p (g c) -> p g c", g=32)[:, :, 0:NB],
                )

            # ---------- phase C: per-head attention ----------
            outT_sb = [opool.tile([128, T], f32r, tag=f"oT{ct}", name=f"oT{ct}")
                       for ct in range(2)]
            ds128 = opool.tile([128, NHEADS * 8], f32, tag="ds")
            rcp = opool.tile([128, NHEADS * 8], f32, tag="rcp")
            rec2 = [opool.tile([2, T], f32, tag=f"rec{ct}", name=f"rec{ct}")
                    for ct in range(2)]
            pvraw = []
            vaug_sb = []
            # all band-bias windows up front (only need sbf_all); keeps the
            # gpsimd queue ahead of the exp stream
            wins = {}
            for st in range(NT):
                for h in range(NHEADS):
                    w_ = bpool.tile([128, 136], bf16, tag="win", bufs=32,
                                    name=f"win{h}_{st}")
                    nc.gpsimd.local_scatter(
                        w_[:],
                        sbf_all[:, (st * 4 + h) * 10:(st * 4 + h) * 10 + 10],
                        sidx_sb[:], channels=128, num_elems=136, num_idxs=10,
                    )
                    wins[(h, st)] = w_
            abs4 = [bpool.tile([128, 16], bf16, tag=f"abs{st}",
                               name=f"abs4_{st}") for st in range(NT)]
            with tc.tile_pool(name="psC", bufs=1, space="PSUM") as psC:

                def scores_tile(h, st):
                    ct, r0 = h // 2, (h % 2) * 64
                    s0 = st * 128
                    sc = psC.tile([128, T], f32, tag="sc", bufs=2, name="sc")
                    for nh in range(2):
                        mmr(
                            sc[:, nh * 512:(nh + 1) * 512],
                            kT_sb[ct][r0:r0 + 64, s0:s0 + 128],
                            qsT_sb[ct][r0:r0 + 64, nh * 512:(nh + 1) * 512],
                            start=True, stop=True,
                        )
                    lo = 4 if st == 0 else 0
                    hi = 132 if st == NT - 1 else 136
                    c = lo
                    while c < hi:
                        col = s0 - 4 + c
                        nxt = min(hi, c + (512 - (col % 512)))
                        nc.tensor.matmul(
                            sc[:, col:col + (nxt - c)],
                            identb[:], wins[(h, st)][:, c:nxt],
                            start=False, stop=True, skip_group_check=True,
                        )
                        c = nxt
                    et = etpool.tile([128, T], bf16, tag="et", name="et")
                    nc.scalar.activation(et[:], sc[:], Exp)
                    # band diagonals -> abs4[p, j] = et[p, s0-4+p+j]
                    if st == 0:
                        nc.gpsimd.local_scatter(
                            abs4[st][:], et[:, 0:136], sidxA_sb[:],
                            channels=128, num_elems=16, num_idxs=136,
                        )
                    else:
                        w = 132 if st == NT - 1 else 136
                        nc.gpsimd.local_scatter(
                            abs4[st][:], et[:, s0 - 4:s0 - 4 + w],
                            sidxB_sb[:, 0:w],
                            channels=128, num_elems=16, num_idxs=w,
                        )
                    return et

                def pv_accum(pv, h, st, et):
                    for nh in range(2):
                        nc.tensor.matmul(
                            pv[:, nh * 512:(nh + 1) * 512],
                            vaug_sb[st][:, h * 65:h * 65 + 65],
                            et[:, nh * 512:(nh + 1) * 512],
                            start=(st == 0), stop=False,
                        )

                def head_tail(pv, h):
                    # rel-V: transpose band diagonals to [j, s] layout, then
                    # the s -> t = s-4+j shift via the atd DRAM pitch trick
                    pat = psC.tile([16, T], bf16, tag="pat", bufs=1, name="pat")
                    for st in range(NT):
                        trp(pat[:, st * 128:(st + 1) * 128], abs4[st][:],
                            identb[:])
                    at16 = bpool.tile([16, T], bf16, tag=f"at{h % 2}",
                                      name=f"at16_{h}")
                    nc.vector.tensor_copy(at16[:], pat[:])
                    nc.sync.dma_start(
                        bass.AP(atd[:].tensor, atd[:].offset + 4,
                                [[RLDW, NB], [1, T]]),
                        at16[0:NB, :],
                    )
                    abt = bpool.tile([NB, T], bf16, tag=f"abt{h % 2}",
                                     name=f"abt{h}")
                    nc.sync.dma_start(
                        abt[:],
                        bass.AP(atd[:].tensor, atd[:].offset + 8,
                                [[T + 7, NB], [1, T]]),
                    )
                    for nh in range(2):
                        nc.tensor.matmul(
                            pv[:, nh * 512:(nh + 1) * 512],
                            ev_sb[:],
                            abt[:, nh * 512:(nh + 1) * 512],
                            start=False, stop=(nh == 1),
                        )
                    # evacuate raw pv; row 64 holds the softmax denominator
                    pvr = opool.tile([D + 1, T], f32, tag=f"pvr{h}",
                                     name=f"pvr{h}")
                    nc.vector.tensor_copy(pvr[:], pv[:])
                    pvraw.append(pvr)
                    # denominators -> [128, 8] layout: ds128[p, h*8+c] = d[8p+c]
                    nc.sync.dma_start(
                        ds128[:, h * 8:h * 8 + 8], pvr[D:D + 1, :],
                    )
                    if h % 2 == 1:
                        pr = h // 2
                        nc.vector.reciprocal(
                            rcp[:, pr * 16:pr * 16 + 16],
                            ds128[:, pr * 16:pr * 16 + 16],
                        )
                        for hh in range(2):
                            nc.sync.dma_start(
                                rec2[pr][hh:hh + 1, :],
                                rcp[:, (2 * pr + hh) * 8:(2 * pr + hh) * 8 + 8],
                            )
                        # broadcast 1/d across the pair's 128 partitions
                        rb = psC.tile([128, T], f32, tag="sc", bufs=2,
                                      name="rb")
                        for nh in range(2):
                            mmr(rb[:, nh * 512:(nh + 1) * 512],
                                e2_sb[:],
                                rec2[pr][:, nh * 512:(nh + 1) * 512],
                                start=True, stop=True)
                        for hh in range(2):
                            hq, rq = 2 * pr + hh, hh * 64
                            nc.vector.tensor_tensor(
                                outT_sb[pr][rq:rq + 64, :],
                                pvraw[hq][0:D, :],
                                rb[rq:rq + 64, :], op=AluMult,
                            )

                # head 0: interleave scores / v projection / PV per tile
                pv0 = psC.tile([D + 1, T], f32, tag="pv", bufs=1, name="pv0")
                for st in range(NT):
                    et = scores_tile(0, st)
                    va = vpool.tile([128, NHEADS * (D + 1)], bf16,
                                    tag=f"va{st}", name=f"va{st}")
                    nc.gpsimd.memset(
                        va[:].rearrange("p (h c) -> p h c", h=NHEADS)
                        [:, :, D:D + 1], 1.0,
                    )
                    ps = psC.tile([128, CH], f32, tag="v", bufs=1, name="psv")
                    for kt in range(4):
                        mmr(
                            ps[:],
                            cT_sb[kt][:, st * 128:(st + 1) * 128],
                            wv_sb[kt][:],
                            start=(kt == 0), stop=False,
                        )
                    mmr(ps[:], ones1[:], bv_sb[:], start=False, stop=True)
                    nc.vector.tensor_copy(
                        va[:].rearrange("p (h c) -> p h c", h=NHEADS)[:, :, 0:D],
                        ps[:].rearrange("p (h c) -> p h c", h=NHEADS),
                    )
                    vaug_sb.append(va)
                    pv_accum(pv0, 0, st, et)
                head_tail(pv0, 0)

                for h in range(1, NHEADS):
                    pv = psC.tile([D + 1, T], f32, tag="pv", bufs=1,
                                  name=f"pv{h}")
                    for st in range(NT):
                        et = scores_tile(h, st)
                        pv_accum(pv, h, st, et)
                    head_tail(pv, h)

            # ---------- phase D: output projection (heads pre-normalized) ----
            with tc.tile_pool(name="psP", bufs=2, space="PSUM") as psP:
                for st in range(NT):
                    pp = psP.tile([128, CIN], f32, tag="pj")
                    for ct in range(2):
                        mmr(
                            pp[:],
                            outT_sb[ct][:, st * 128:(st + 1) * 128],
                            wo_sb[ct][:],
                            start=(ct == 0), stop=(ct == 1),
                        )
                    acc = opool.tile([128, CIN], f32, tag=f"acc{st % 2}")
                    if st % 2 == 0:
                        nc.scalar.activation(acc[:], pp[:], Copy)
                    else:
                        nc.vector.tensor_copy(acc[:], pp[:])
                    nc.sync.dma_start(out_p[st * 128:(st + 1) * 128, :], acc[:])

    nc.compile()
    return nc


def make_core_inputs(x, c, Wq, bq, Wk, bk, Wv, bv, Wo, bo, emb_rel_k, emb_rel_v,
                     core):
    b, hg = core // 2, core % 2
    sl = slice(hg * CH, (hg + 1) * CH)
    ek2p = np.zeros((128, 32 + NB), np.float32)
    ek2p[0:D, 0:NB] = emb_rel_k[0].T
    ek2p[D:2 * D, 32:32 + NB] = emb_rel_k[0].T
    e2p = np.zeros((2, 128), np.float32)
    e2p[0, 0:64] = 1.0
    e2p[1, 64:128] = 1.0
    ev65 = np.zeros((NB, D + 1), np.float32)
    ev65[:, 0:D] = emb_rel_v[0][::-1]
    si = np.zeros((128, 10), np.int16)
    siA = np.full((128, 136), -1, np.int16)
    siB = np.full((128, 136), -1, np.int16)
    for p in range(128):
        for cc in range(136):
            j = cc + 4 - p
            if 0 <= j < NB:
                siA[p, cc] = j
            j = cc - p
            if 0 <= j < NB:
                siB[p, cc] = j
    for p in range(128):
        for j in range(NB):
            si[p, j] = p + 8 - j
        si[p, 9] = -1
    return {
        "xT": np.ascontiguousarray(x[b].T).astype(np.float32),
        "cT": np.ascontiguousarray(c[b].T).astype(np.float32),
        "wq": np.ascontiguousarray(Wq[:, sl]).astype(np.float32),
        "wk": np.ascontiguousarray(Wk[:, sl]).astype(np.float32),
        "wv": np.ascontiguousarray(Wv[:, sl]).astype(np.float32),
        "wo": np.ascontiguousarray(Wo[sl, :]).astype(np.float32),
        "bq2": np.ascontiguousarray((bq[sl] * 0.125).reshape(2, 128).T).astype(np.float32),
        "bk2": np.ascontiguousarray(bk[sl].reshape(2, 128).T).astype(np.float32),
        "bv1": bv[sl].reshape(1, CH).astype(np.float32),
        "ek2p": ek2p,
        "ev65": ev65,
        "ones128": np.ones((1, 128), np.float32),
        "e2p": e2p,
        "sidx": si,
        "sidxA": siA,
        "sidxB": siB,
    }


def kernel(**inputs):
    inputs = {k: np.asarray(v) for k, v in inputs.items()}
    nc = build_program()
    core_ids = list(range(8))
    in_maps = [make_core_inputs(core=i, **inputs) for i in core_ids]
    res = run_bass_kernel_spmd(nc, in_maps, core_ids).results
    B = inputs["x"].shape[0]
    out = np.zeros((B, T, CIN), np.float32)
    for b in range(B):
        out[b] = res[2 * b]["out_p"] + res[2 * b + 1]["out_p"] + inputs["bo"]
    return out


# revision 16
# speedup vs baseline: 1.0775x; 1.0775x over previous
"""Trainium2 Bass kernel for VITS-style relative-position MultiHeadAttention.

Problem: B=4, T=1024, C=512, H=8 heads, d=64, window=4 relative attention
(rel embeddings shared across heads). Sharded over 8 NeuronCores as
(batch x head-group): core = 2*b + hg, each core handles batch b and 4 heads.

v4 changes vs v3 (trace-driven):
  - warmup matmul burst + exp-table preload at t=0 (HAM stays at K=8/8)
  - DMA order wq+xT first; q projection starts as tiles land (kt-outer),
    rel-K skew bounce batched into 8 DMAs on the scalar HWDGE ring and
    overlapped with the k/v projections (was a 16.8us PE stall)
  - et / vaug / G-band storage in bf16 (halves band DMA + DVE traffic)
  - rel-V band read (abt) pulled straight out of the skewed G buffer with
    a diagonal access pattern: the per-head abs4 readback, PE transposes,
    at_cat copy and atd re-skew of v3 are gone entirely
  - softmax reciprocal on [128,16]-shaped tiles via SBUF->SBUF DMA
    reshape (was 7.8us per pair on [2,1024] = 2 DVE lanes)
  - pair-0 normalization + outT multiply run mid-flight under heads 2/3
"""

import numpy as np

import concourse.bass as bass
import concourse.bacc as bacc
import concourse.mybir as mybir
import concourse.tile as tile
from concourse.bass_utils import run_bass_kernel_spmd
from concourse.masks import make_identity

f32 = mybir.dt.float32
f32r = mybir.dt.float32r
bf16 = mybir.dt.bfloat16
i16 = mybir.dt.int16

T = 1024          # sequence length (t_t == t_s)
CIN = 512         # input channels
CH = 256          # channels per core (head group)
NHEADS = 4        # heads per core
D = 64            # head dim
NB = 9            # band width (2*window+1)
NT = T // 128     # 8 tiles of 128
GPITCH = 137      # G buffer row pitch (136 + 1)
GBASE = 4 * GPITCH          # origin shift: row s lives at GBASE + s*GPITCH
GSZ = (T + 8) * GPITCH + 32  # rows -4 .. 1027 plus slack
RLDW = T + 8      # rld row pitch

Exp = mybir.ActivationFunctionType.Exp
Identity = mybir.ActivationFunctionType.Identity
Copy = mybir.ActivationFunctionType.Copy
AluAdd = mybir.AluOpType.add
AluMult = mybir.AluOpType.mult


def build_program():
    nc = bacc.Bacc()

    # fp32r matmul: fp32 data, 1 PE cycle/row when moving dim >= 256
    def mmr(out, lhsT, rhs, **kw):
        nc.tensor.matmul(out, lhsT.bitcast(f32r), rhs.bitcast(f32r), **kw)

    def trp(out, in_, identity):
        nc.tensor.matmul(out, in_, identity, is_transpose=True)

    # ---- external I/O (per-core shapes) ----
    xT = nc.declare_dram_parameter("xT", [CIN, T], f32r, isOutput=False)
    cT = nc.declare_dram_parameter("cT", [CIN, T], f32r, isOutput=False)
    wq = nc.declare_dram_parameter("wq", [CIN, CH], f32r, isOutput=False)
    wk = nc.declare_dram_parameter("wk", [CIN, CH], f32r, isOutput=False)
    wv = nc.declare_dram_parameter("wv", [CIN, CH], f32r, isOutput=False)
    wo = nc.declare_dram_parameter("wo", [CH, CIN], f32r, isOutput=False)
    bq2 = nc.declare_dram_parameter("bq2", [128, 2], f32, isOutput=False)
    bk2 = nc.declare_dram_parameter("bk2", [128, 2], f32, isOutput=False)
    bv1 = nc.declare_dram_parameter("bv1", [1, CH], f32r, isOutput=False)
    ek2p = nc.declare_dram_parameter("ek2p", [128, 32 + NB], f32r, isOutput=False)
    ev65 = nc.declare_dram_parameter("ev65", [NB, D + 1], f32, isOutput=False)
    ones128 = nc.declare_dram_parameter("ones128", [1, 128], f32r, isOutput=False)
    e2p = nc.declare_dram_parameter("e2p", [2, 128], f32r, isOutput=False)
    sidx = nc.declare_dram_parameter("sidx", [128, 10], i16, isOutput=False)
    sidxA = nc.declare_dram_parameter("sidxA", [128, 136], i16, isOutput=False)
    sidxB = nc.declare_dram_parameter("sidxB", [128, 136], i16, isOutput=False)
    out_p = nc.declare_dram_parameter("out_p", [T, CIN], f32, isOutput=True)

    with tile.TileContext(nc) as tc:
        with (
            tc.tile_pool(name="const", bufs=1) as cpool,
            tc.tile_pool(name="win", bufs=1) as wpool,
            tc.tile_pool(name="xin", bufs=1) as xpool,
            tc.tile_pool(name="qk", bufs=1) as qkpool,
            tc.tile_pool(name="vaug", bufs=1) as vpool,
            tc.tile_pool(name="band", bufs=1) as bpool,
            tc.tile_pool(name="et", bufs=8) as etpool,
            tc.tile_pool(name="outp", bufs=1) as opool,
            tc.tile_pool(name="dram", bufs=1, space="DRAM") as dpool,
        ):
            # ---------- constants (scalar HWDGE ring) ----------
            ident = cpool.tile([128, 128], f32)
            make_identity(nc, ident[:])
            identb = cpool.tile([128, 128], bf16)
            make_identity(nc, identb[:])
            ones1 = cpool.tile([1, 128], f32r)
            nc.scalar.dma_start(ones1[:], ones128[:])
            e2_sb = cpool.tile([2, 128], f32r)
            nc.scalar.dma_start(e2_sb[:], e2p[:])
            sidx_sb = cpool.tile([128, 10], i16)
            nc.scalar.dma_start(sidx_sb[:], sidx[:])
            sidxA_sb = cpool.tile([128, 136], i16)
            nc.scalar.dma_start(sidxA_sb[:], sidxA[:])
            sidxB_sb = cpool.tile([128, 136], i16)
            nc.scalar.dma_start(sidxB_sb[:], sidxB[:])
            ek2f = cpool.tile([128, 32 + NB], f32r)
            nc.scalar.dma_start(ek2f[:], ek2p[:])
            ek2 = cpool.tile([128, 32 + NB], bf16)
            nc.vector.tensor_copy(ek2[:], ek2f[:].bitcast(f32))
            ev_f = cpool.tile([NB, D + 1], f32)
            nc.scalar.dma_start(ev_f[:], ev65[:])
            ev_sb = cpool.tile([NB, D + 1], bf16)
            nc.vector.tensor_copy(ev_sb[:], ev_f[:])
            bq_sb = cpool.tile([128, 2], f32)
            nc.scalar.dma_start(bq_sb[:], bq2[:])
            bk_sb = cpool.tile([128, 2], f32)
            nc.scalar.dma_start(bk_sb[:], bk2[:])
            bv_sb = cpool.tile([1, CH], f32r)
            nc.scalar.dma_start(bv_sb[:], bv1[:])
            zb16 = cpool.tile([NB, 8], bf16)
            nc.gpsimd.memset(zb16[:], 0.0)
            zb_f = cpool.tile([36, 8], f32)
            nc.gpsimd.memset(zb_f[:], 0.0)

            # rld bounce (rel-K skew) borders: cols 0..3 and T+4..T+7 of all
            # 36 rows
            rld = dpool.tile([1, 36 * RLDW], f32, name="rld")
            nc.scalar.dma_start(
                bass.AP(rld[:].tensor, rld[:].offset,
                        [[RLDW, 36], [T + 4, 2], [1, 4]]),
                bass.AP(zb_f[:].tensor, zb_f[:].offset, [[8, 36], [4, 2], [1, 4]]),
            )
            # atd bounce (rel-V skew), bf16, rows j=0..8 pitch T+8
            atd = dpool.tile([1, NB * RLDW], bf16, name="atd")
            nc.scalar.dma_start(
                bass.AP(atd[:].tensor, atd[:].offset,
                        [[RLDW, NB], [T + 4, 2], [1, 4]]),
                bass.AP(zb16[:].tensor, zb16[:].offset, [[8, NB], [4, 2], [1, 4]]),
            )

            # ---------- PE warmup (HAM) + ACT exp-table preload ----------
            wact = cpool.tile([1, 2], f32)
            nc.scalar.activation(wact[0:1, 0:1], ident[0:1, 0:1], Exp)
            with tc.tile_pool(name="psW", bufs=1, space="PSUM") as psW:
                wps = psW.tile([128, 128], f32)
                for _ in range(8):
                    nc.tensor.matmul(wps[:], ident[:], ident[:],
                                     start=True, stop=True)

            # ---------- input loads (sync ring), q-critical first ----------
            wq_sb = []
            xT_sb = []
            for kt in range(4):
                t_ = wpool.tile([128, CH], f32r, tag=f"wq{kt}")
                nc.sync.dma_start(t_[:], wq[kt * 128:(kt + 1) * 128, :])
                wq_sb.append(t_)
                t_ = xpool.tile([128, T], f32r, tag=f"xT{kt}")
                nc.sync.dma_start(t_[:], xT[kt * 128:(kt + 1) * 128, :])
                xT_sb.append(t_)
            wk_sb = []
            cT_sb = []
            for kt in range(4):
                t_ = wpool.tile([128, CH], f32r, tag=f"wk{kt}")
                nc.scalar.dma_start(t_[:], wk[kt * 128:(kt + 1) * 128, :])
                wk_sb.append(t_)
                t_ = xpool.tile([128, T], f32r, tag=f"cT{kt}")
                nc.scalar.dma_start(t_[:], cT[kt * 128:(kt + 1) * 128, :])
                cT_sb.append(t_)
            wv_sb = []
            for kt in range(4):
                t_ = wpool.tile([128, CH], f32r, tag=f"wv{kt}")
                nc.sync.dma_start(t_[:], wv[kt * 128:(kt + 1) * 128, :])
                wv_sb.append(t_)
            wo_sb = []
            for ct in range(2):
                t_ = wpool.tile([128, CIN], f32r, tag=f"wo{ct}")
                nc.sync.dma_start(t_[:], wo[ct * 128:(ct + 1) * 128, :])
                wo_sb.append(t_)

            # band-prep SBUF tiles (memset before skew readback writes rows)
            rlp_cat = bpool.tile([128, T], f32, tag="rlpc")
            s4t_cat = bpool.tile([64, T], f32, tag="s4t")
            nc.gpsimd.memset(s4t_cat[:], 0.0)
            sbf_all = bpool.tile([128, NT * NHEADS * 10], bf16, tag="sbfall")
            nc.gpsimd.memset(sbf_all[:], 0.0)

            qsT_sb = [qkpool.tile([128, T], bf16, tag=f"qsT{ct}", name=f"qsT{ct}")
                      for ct in range(2)]
            kT_sb = [qkpool.tile([128, T], bf16, tag=f"kT{ct}", name=f"kT{ct}")
                     for ct in range(2)]

            with tc.tile_pool(name="psAB", bufs=1, space="PSUM") as psAB:
                # ---- q projection, kt-outer so matmuls start on first tiles
                psq = {}
                for ct in range(2):
                    for nh in range(2):
                        psq[(ct, nh)] = psAB.tile(
                            [128, 512], f32, tag=f"q{ct}{nh}", bufs=1,
                            name=f"psq{ct}{nh}")
                for kt in range(4):
                    for ct in range(2):
                        for nh in range(2):
                            mmr(
                                psq[(ct, nh)][:],
                                wq_sb[kt][:, ct * 128:(ct + 1) * 128],
                                xT_sb[kt][:, nh * 512:(nh + 1) * 512],
                                start=(kt == 0), stop=(kt == 3),
                            )
                for ct in range(2):
                    for nh in range(2):
                        # q_scaled = (x@Wq)*0.125 + bq*0.125 (bq2 pre-scaled)
                        nc.scalar.activation(
                            qsT_sb[ct][:, nh * 512:(nh + 1) * 512],
                            psq[(ct, nh)][:],
                            Identity, bias=bq_sb[:, ct:ct + 1], scale=0.125,
                        )

                # ---- rel-K band logits + batched skew bounce (scalar ring)
                for ct in range(2):
                    for nh in range(2):
                        rlt = psAB.tile([41, 512], f32, tag="small", bufs=2)
                        nc.tensor.matmul(
                            rlt[:], ek2[:],
                            qsT_sb[ct][:, nh * 512:(nh + 1) * 512],
                            start=True, stop=True)
                        for hh in range(2):
                            h = 2 * ct + hh
                            nc.vector.tensor_copy(
                                rlp_cat[h * 32:h * 32 + NB,
                                        nh * 512:(nh + 1) * 512],
                                rlt[hh * 32:hh * 32 + NB, :],
                            )
                for h in range(NHEADS):
                    nc.sync.dma_start(
                        bass.AP(rld[:].tensor,
                                rld[:].offset + h * 9 * RLDW + 4,
                                [[RLDW, NB], [1, T]]),
                        rlp_cat[h * 32:h * 32 + NB, :],
                    )
                for h in range(NHEADS):
                    # s4t'[h*16+r, c] = rld[h*9+r, 8-r+c] (pitch T+7 re-read)
                    nc.sync.dma_start(
                        s4t_cat[h * 16:h * 16 + NB, :],
                        bass.AP(rld[:].tensor,
                                rld[:].offset + h * 9 * RLDW + 8,
                                [[T + 7, NB], [1, T]]),
                    )

                # ---- k projection (reuses the q psum tags)
                psk = {}
                for ct in range(2):
                    for nh in range(2):
                        psk[(ct, nh)] = psAB.tile(
                            [128, 512], f32, tag=f"q{ct}{nh}", bufs=1,
                            name=f"psk{ct}{nh}")
                for kt in range(4):
                    for ct in range(2):
                        for nh in range(2):
                            mmr(
                                psk[(ct, nh)][:],
                                wk_sb[kt][:, ct * 128:(ct + 1) * 128],
                                cT_sb[kt][:, nh * 512:(nh + 1) * 512],
                                start=(kt == 0), stop=(kt == 3),
                            )
                for ct in range(2):
                    for nh in range(2):
                        nc.vector.tensor_scalar(
                            kT_sb[ct][:, nh * 512:(nh + 1) * 512],
                            psk[(ct, nh)][:],
                            bk_sb[:, ct:ct + 1], None, op0=AluAdd,
                        )

                # ---- transpose skewed rel-K logits into S layout
                pst = psAB.tile([128, 512], f32, tag="small", bufs=2)
                for st in range(NT):
                    trp(
                        pst[:, st * 64:(st + 1) * 64],
                        s4t_cat[:, st * 128:(st + 1) * 128],
                        ident[0:64, 0:64],
                    )
                nc.vector.tensor_copy(
                    sbf_all[:].rearrange("p (g c) -> p g c", g=32)[:, :, 0:NB],
                    pst[:].rearrange("p (g c) -> p g c", g=32)[:, :, 0:NB],
                )

            # ---------- phase C: per-head attention ----------
            outT_sb = [opool.tile([128, T], f32r, tag=f"oT{ct}", name=f"oT{ct}")
                       for ct in range(2)]
            ds128 = opool.tile([128, NHEADS * 8], f32, tag="ds")
            rcp = opool.tile([128, NHEADS * 8], f32, tag="rcp")
            rec2 = [opool.tile([2, T], f32, tag=f"rec{ct}", name=f"rec{ct}")
                    for ct in range(2)]
            pvraw = []
            vaug_sb = []
            # all band-bias windows up front (only need sbf_all); keeps the
            # gpsimd queue ahead of the exp stream
            wins = {}
            for st in range(NT):
                for h in range(NHEADS):
                    w_ = bpool.tile([128, 136], bf16, tag="win", bufs=32,
                                    name=f"win{h}_{st}")
                    nc.gpsimd.local_scatter(
                        w_[:],
                        sbf_all[:, (st * 4 + h) * 10:(st * 4 + h) * 10 + 10],
                        sidx_sb[:], channels=128, num_elems=136, num_idxs=10,
                    )
                    wins[(h, st)] = w_
            abs4 = [bpool.tile([128, 16], bf16, tag=f"abs{st}",
                               name=f"abs4_{st}") for st in range(NT)]
            with tc.tile_pool(name="psC", bufs=1, space="PSUM") as psC:

                def scores_tile(h, st):
                    ct, r0 = h // 2, (h % 2) * 64
                    s0 = st * 128
                    sc = psC.tile([128, T], f32, tag="sc", bufs=2, name="sc")
                    for nh in range(2):
                        nc.tensor.matmul(
                            sc[:, nh * 512:(nh + 1) * 512],
                            kT_sb[ct][r0:r0 + 64, s0:s0 + 128],
                            qsT_sb[ct][r0:r0 + 64, nh * 512:(nh + 1) * 512],
                            start=True, stop=True,
                        )
                    lo = 4 if st == 0 else 0
                    hi = 132 if st == NT - 1 else 136
                    c = lo
                    while c < hi:
                        col = s0 - 4 + c
                        nxt = min(hi, c + (512 - (col % 512)))
                        nc.tensor.matmul(
                            sc[:, col:col + (nxt - c)],
                            identb[:], wins[(h, st)][:, c:nxt],
                            start=False, stop=True, skip_group_check=True,
                        )
                        c = nxt
                    et = etpool.tile([128, T], bf16, tag="et", name="et")
                    nc.scalar.activation(et[:], sc[:], Exp)
                    # band diagonals -> abs4[p, j] = et[p, s0-4+p+j]
                    if st == 0:
                        nc.gpsimd.local_scatter(
                            abs4[st][:], et[:, 0:136], sidxA_sb[:],
                            channels=128, num_elems=16, num_idxs=136,
                        )
                    else:
                        w = 132 if st == NT - 1 else 136
                        nc.gpsimd.local_scatter(
                            abs4[st][:], et[:, s0 - 4:s0 - 4 + w],
                            sidxB_sb[:, 0:w],
                            channels=128, num_elems=16, num_idxs=w,
                        )
                    return et

                def pv_accum(pv, h, st, et):
                    for nh in range(2):
                        nc.tensor.matmul(
                            pv[:, nh * 512:(nh + 1) * 512],
                            vaug_sb[st][:, h * 65:h * 65 + 65],
                            et[:, nh * 512:(nh + 1) * 512],
                            start=(st == 0), stop=False,
                        )

                def head_tail(pv, h):
                    # rel-V: transpose band diagonals to [j, s] layout, then
                    # the s -> t = s-4+j shift via the atd DRAM pitch trick
                    pat = psC.tile([16, T], bf16, tag="pat", bufs=1, name="pat")
                    for st in range(NT):
                        trp(pat[:, st * 128:(st + 1) * 128], abs4[st][:],
                            identb[:])
                    at16 = bpool.tile([16, T], bf16, tag=f"at{h % 2}",
                                      name=f"at16_{h}")
                    nc.vector.tensor_copy(at16[:], pat[:])
                    nc.sync.dma_start(
                        bass.AP(atd[:].tensor, atd[:].offset + 4,
                                [[RLDW, NB], [1, T]]),
                        at16[0:NB, :],
                    )
                    abt = bpool.tile([NB, T], bf16, tag=f"abt{h % 2}",
                                     name=f"abt{h}")
                    nc.sync.dma_start(
                        abt[:],
                        bass.AP(atd[:].tensor, atd[:].offset + 8,
                                [[T + 7, NB], [1, T]]),
                    )
                    for nh in range(2):
                        nc.tensor.matmul(
                            pv[:, nh * 512:(nh + 1) * 512],
                            ev_sb[:],
                            abt[:, nh * 512:(nh + 1) * 512],
                            start=False, stop=(nh == 1),
                        )
                    # evacuate raw pv; row 64 holds the softmax denominator
                    pvr = opool.tile([D + 1, T], f32, tag=f"pvr{h}",
                                     name=f"pvr{h}")
                    nc.vector.tensor_copy(pvr[:], pv[:])
                    pvraw.append(pvr)
                    # denominators -> [128, 8] layout: ds128[p, h*8+c] = d[8p+c]
                    nc.sync.dma_start(
                        ds128[:, h * 8:h * 8 + 8], pvr[D:D + 1, :],
                    )
                    if h % 2 == 1:
                        pr = h // 2
                        nc.vector.reciprocal(
                            rcp[:, pr * 16:pr * 16 + 16],
                            ds128[:, pr * 16:pr * 16 + 16],
                        )
                        for hh in range(2):
                            nc.sync.dma_start(
                                rec2[pr][hh:hh + 1, :],
                                rcp[:, (2 * pr + hh) * 8:(2 * pr + hh) * 8 + 8],
                            )
                        # broadcast 1/d across the pair's 128 partitions
                        rb = psC.tile([128, T], f32, tag="sc", bufs=2,
                                      name="rb")
                        for nh in range(2):
                            mmr(rb[:, nh * 512:(nh + 1) * 512],
                                e2_sb[:],
                                rec2[pr][:, nh * 512:(nh + 1) * 512],
                                start=True, stop=True)
                        for hh in range(2):
                            hq, rq = 2 * pr + hh, hh * 64
                            nc.vector.tensor_tensor(
                                outT_sb[pr][rq:rq + 64, :],
                                pvraw[hq][0:D, :],
                                rb[rq:rq + 64, :], op=AluMult,
                            )

                # head 0: interleave scores / v projection / PV per tile
                pv0 = psC.tile([D + 1, T], f32, tag="pv", bufs=1, name="pv0")
                for st in range(NT):
                    et = scores_tile(0, st)
                    va = vpool.tile([128, NHEADS * (D + 1)], bf16,
                                    tag=f"va{st}", name=f"va{st}")
                    nc.gpsimd.memset(
                        va[:].rearrange("p (h c) -> p h c", h=NHEADS)
                        [:, :, D:D + 1], 1.0,
                    )
                    ps = psC.tile([128, CH], f32, tag="v", bufs=1, name="psv")
                    for kt in range(4):
                        mmr(
                            ps[:],
                            cT_sb[kt][:, st * 128:(st + 1) * 128],
                            wv_sb[kt][:],
                            start=(kt == 0), stop=False,
                        )
                    mmr(ps[:], ones1[:], bv_sb[:], start=False, stop=True)
                    nc.vector.tensor_copy(
                        va[:].rearrange("p (h c) -> p h c", h=NHEADS)[:, :, 0:D],
                        ps[:].rearrange("p (h c) -> p h c", h=NHEADS),
                    )
                    vaug_sb.append(va)
                    pv_accum(pv0, 0, st, et)
                head_tail(pv0, 0)

                for h in range(1, NHEADS):
                    pv = psC.tile([D + 1, T], f32, tag="pv", bufs=1,
                                  name=f"pv{h}")
                    for st in range(NT):
                        et = scores_tile(h, st)
                        pv_accum(pv, h, st, et)
                    head_tail(pv, h)

            # ---------- phase D: output projection (heads pre-normalized) ----
            with tc.tile_pool(name="psP", bufs=2, space="PSUM") as psP:
                for st in range(NT):
                    pp = psP.tile([128, CIN], f32, tag="pj")
                    for ct in range(2):
                        mmr(
                            pp[:],
                            outT_sb[ct][:, st * 128:(st + 1) * 128],
                            wo_sb[ct][:],
                            start=(ct == 0), stop=(ct == 1),
                        )
                    acc = opool.tile([128, CIN], f32, tag=f"acc{st % 2}")
                    if st % 2 == 0:
                        nc.scalar.activation(acc[:], pp[:], Copy)
                    else:
                        nc.vector.tensor_copy(acc[:], pp[:])
                    nc.sync.dma_start(out_p[st * 128:(st + 1) * 128, :], acc[:])

    nc.compile()
    return nc


def make_core_inputs(x, c, Wq, bq, Wk, bk, Wv, bv, Wo, bo, emb_rel_k, emb_rel_v,
                     core):
    b, hg = core // 2, core % 2
    sl = slice(hg * CH, (hg + 1) * CH)
    ek2p = np.zeros((128, 32 + NB), np.float32)
    ek2p[0:D, 0:NB] = emb_rel_k[0].T
    ek2p[D:2 * D, 32:32 + NB] = emb_rel_k[0].T
    e2p = np.zeros((2, 128), np.float32)
    e2p[0, 0:64] = 1.0
    e2p[1, 64:128] = 1.0
    ev65 = np.zeros((NB, D + 1), np.float32)
    ev65[:, 0:D] = emb_rel_v[0][::-1]
    si = np.zeros((128, 10), np.int16)
    siA = np.full((128, 136), -1, np.int16)
    siB = np.full((128, 136), -1, np.int16)
    for p in range(128):
        for cc in range(136):
            j = cc + 4 - p
            if 0 <= j < NB:
                siA[p, cc] = j
            j = cc - p
            if 0 <= j < NB:
                siB[p, cc] = j
    for p in range(128):
        for j in range(NB):
            si[p, j] = p + 8 - j
        si[p, 9] = -1
    return {
        "xT": np.ascontiguousarray(x[b].T).astype(np.float32),
        "cT": np.ascontiguousarray(c[b].T).astype(np.float32),
        "wq": np.ascontiguousarray(Wq[:, sl]).astype(np.float32),
        "wk": np.ascontiguousarray(Wk[:, sl]).astype(np.float32),
        "wv": np.ascontiguousarray(Wv[:, sl]).astype(np.float32),
        "wo": np.ascontiguousarray(Wo[sl, :]).astype(np.float32),
        "bq2": np.ascontiguousarray((bq[sl] * 0.125).reshape(2, 128).T).astype(np.float32),
        "bk2": np.ascontiguousarray(bk[sl].reshape(2, 128).T).astype(np.float32),
        "bv1": bv[sl].reshape(1, CH).astype(np.float32),
        "ek2p": ek2p,
        "ev65": ev65,
        "ones128": np.ones((1, 128), np.float32),
        "e2p": e2p,
        "sidx": si,
        "sidxA": siA,
        "sidxB": siB,
    }


def kernel(**inputs):
    inputs = {k: np.asarray(v) for k, v in inputs.items()}
    nc = build_program()
    core_ids = list(range(8))
    in_maps = [make_core_inputs(core=i, **inputs) for i in core_ids]
    res = run_bass_kernel_spmd(nc, in_maps, core_ids).results
    B = inputs["x"].shape[0]
    out = np.zeros((B, T, CIN), np.float32)
    for b in range(B):
        out[b] = res[2 * b]["out_p"] + res[2 * b + 1]["out_p"] + inputs["bo"]
    return out
